# revision 5
# baseline (speedup 1.0000x reference)
"""AdaDualFocal loss on 8 TRN2 NeuronCores — data-parallel raw-Bass kernel.

Math: per row i (C classes), with k = target[i]:
  s   = sum_j exp(x_ij)                      (softmax denominator, no max-shift:
                                              inputs are randn, exp(max) ~ 300, safe in f32)
  e_k = exp(x_ik);  p_k = e_k / s;  logp_k = x_ik - ln(s)
  r   = max_j ( exp(x_ij) * [x_ij < x_ik] )  (largest prob strictly below p_k, times s;
                                              0 if none — matches reference where())
  p_j = r / s;  pt = p_k - p_j
  gamma = bin_gammas[clip(searchsorted(bin_uppers, pt, 'right'), 0, 14)]
        = g0 + sum_b (g[b+1]-g[b]) * [pt >= u_b]   for b in 0..13
  loss_i = -(1 - p_k + p_j)^gamma * logp_k = exp(gamma*ln(1-pt)) * (ln(s) - x_ik)
Output = sum_i loss_i.

Sharding: rows 4096 -> 8 cores x 512 rows. Each core: 4 row-tiles of 128
partitions, streams 32000 columns in chunks; ACT does exp+sum, DVE does the
masked max via scalar_tensor_tensor((x < xk) * e) + reduce_max. Per-core
outputs [128, 12]: per-row losses (4 cols), s (4), r (4); host sums losses.
"""

import os
import numpy as np

import concourse.bass as bass
import concourse.mybir as mybir
from concourse.bass_utils import run_bass_kernel_spmd

N, C, NBINS = 4096, 32000, 15
NCORES = 8
RPC = N // NCORES          # 512 rows per core
P = 128                    # partitions
NT = RPC // P              # 4 row-tiles per core
Q = 4000                   # column chunk width
NCH = C // Q               # 8 chunks per row-tile
NIT = NT * NCH             # 32 hot-loop iterations
XBUF = 3                   # x chunk buffers
EBUF = 2                   # exp chunk buffers

DT = mybir.dt.float32
AF = mybir.ActivationFunctionType
OP = mybir.AluOpType

LAST_EXEC_NS = None
_CACHE = {}


def build(debug=False, reps=1):
    nc = bass.Bass()
    ow = 11 * NT if debug else 3 * NT
    x_ext = nc.declare_dram_parameter("input", [RPC, C], DT, isOutput=False)
    xk_ext = nc.declare_dram_parameter("xk", [P, NT], DT, isOutput=False)
    ub_ext = nc.declare_dram_parameter("ub", [P, NBINS - 1], DT, isOutput=False)
    g0_ext = nc.declare_dram_parameter("g0", [P, 1], DT, isOutput=False)
    dg_ext = nc.declare_dram_parameter("dg", [P, NBINS - 1], DT, isOutput=False)
    out_ext = nc.declare_dram_parameter("out", [P, ow], DT, isOutput=True)

    from contextlib import ExitStack
    with ExitStack() as st:
        sb = lambda name, shape: st.enter_context(nc.sbuf_tensor(name, shape, DT))
        x_bufs = [sb(f"xb{i}", [P, Q]) for i in range(XBUF)]
        e_bufs = [sb(f"eb{i}", [P, Q]) for i in range(EBUF)]
        me = sb("me", [P, Q])
        s_parts = sb("s_parts", [P, NIT])
        r_parts = sb("r_parts", [P, NIT])
        xk = sb("xk_sb", [P, NT])
        ub = sb("ub_sb", [P, NBINS - 1])
        g0 = sb("g0_sb", [P, 1])
        dg = sb("dg_sb", [P, NBINS - 1])
        s4 = sb("s4", [P, NT])
        r4 = sb("r4", [P, NT])
        inv_s = sb("inv_s", [P, NT])
        ls = sb("ls", [P, NT])
        ek = sb("ek", [P, NT])
        p_k = sb("p_k", [P, NT])
        p_j = sb("p_j", [P, NT])
        ptn = sb("ptn", [P, NT])
        q_t = sb("q_t", [P, NT])
        pt = sb("pt", [P, NT])
        gam = sb("gam", [P, NT])
        tmp = sb("tmp", [P, NT])
        lq = sb("lq", [P, NT])
        gl = sb("gl", [P, NT])
        pw = sb("pw", [P, NT])
        nlp = sb("nlp", [P, NT])
        out_t = sb("out_t", [P, ow])

        psem = st.enter_context(nc.semaphore("psem"))
        dsem = st.enter_context(nc.semaphore("dsem"))
        asem = st.enter_context(nc.semaphore("asem"))
        vsem = st.enter_context(nc.semaphore("vsem"))
        esem = st.enter_context(nc.semaphore("esem"))
        osem = st.enter_context(nc.semaphore("osem"))
        block = st.enter_context(nc.Block())

        @block.sync
        def _(sync):
            # params
            sync.dma_start(out=xk[:, :], in_=xk_ext[:, :]).then_inc(psem, 16)
            sync.dma_start(out=ub[:, :], in_=ub_ext[:, :]).then_inc(psem, 16)
            sync.dma_start(out=g0[:, :], in_=g0_ext[:, :]).then_inc(psem, 16)
            sync.dma_start(out=dg[:, :], in_=dg_ext[:, :]).then_inc(psem, 16)
            # chunk stream
            for rep in range(reps):
                for ii in range(NIT):
                    rt, ci = divmod(ii, NCH)
                    g = rep * NIT + ii
                    if g >= XBUF:
                        # slot reuse: DVE stt of iter g-XBUF done implies ACT done
                        sync.wait_ge(vsem, g - XBUF + 1)
                    sync.dma_start(
                        out=x_bufs[g % XBUF][:, :],
                        in_=x_ext[rt * P:(rt + 1) * P, ci * Q:(ci + 1) * Q],
                    ).then_inc(dsem, 16)
            # output
            sync.wait_ge(esem, 7 * reps)
            sync.dma_start(out=out_ext[:, :], in_=out_t[:, :]).then_inc(osem, 16)
            sync.wait_ge(osem, 16)

        @block.scalar
        def _(scalar):
            for rep in range(reps):
                e0 = 7 * rep
                for ii in range(NIT):
                    g = rep * NIT + ii
                    scalar.wait_ge(dsem, 16 * (g + 1))
                    if g >= EBUF:
                        scalar.wait_ge(vsem, g - EBUF + 1)
                    scalar.activation(
                        e_bufs[g % EBUF][:, :], x_bufs[g % XBUF][:, :], AF.Exp,
                        accum_out=s_parts[:, ii:ii + 1],
                    ).then_inc(asem, 1)
                # all accum_out writes drained before DVE reads s_parts tail
                scalar.drain().then_inc(asem, 1)  # -> (rep+1)*(NIT+1)
                # epilogue: ln(s), exp(xk)
                scalar.wait_ge(esem, e0 + 1)
                scalar.activation(ls[:, :], s4[:, :], AF.Ln)
                scalar.activation(ek[:, :], xk[:, :], AF.Exp)
                scalar.drain().then_inc(esem, 1)  # ->2
                scalar.wait_ge(esem, e0 + 3)
                scalar.activation(lq[:, :], q_t[:, :], AF.Ln)
                scalar.drain().then_inc(esem, 1)  # ->4
                scalar.wait_ge(esem, e0 + 5)
                scalar.activation(pw[:, :], gl[:, :], AF.Exp)
                scalar.drain().then_inc(esem, 1)  # ->6

        @block.vector
        def _(vector):
            vector.wait_ge(psem, 64)
            for rep in range(reps):
                e0 = 7 * rep
                for ii in range(NIT):
                    rt = ii // NCH
                    g = rep * NIT + ii
                    vector.wait_ge(asem, rep * (NIT + 1) + ii + 1)
                    vector.scalar_tensor_tensor(
                        out=me[:, :], in0=x_bufs[g % XBUF][:, :],
                        scalar=xk[:, rt:rt + 1], in1=e_bufs[g % EBUF][:, :],
                        op0=OP.is_lt, op1=OP.mult,
                    ).then_inc(vsem, 1)
                    vector.reduce_max(r_parts[:, ii:ii + 1], me[:, :],
                                      axis=mybir.AxisListType.X)
                # finalize row stats
                for rt in range(NT):
                    vector.reduce_max(r4[:, rt:rt + 1],
                                      r_parts[:, rt * NCH:(rt + 1) * NCH],
                                      axis=mybir.AxisListType.X)
                vector.wait_ge(asem, (rep + 1) * (NIT + 1))
                for rt in range(NT):
                    vector.reduce_sum(s4[:, rt:rt + 1],
                                      s_parts[:, rt * NCH:(rt + 1) * NCH],
                                      axis=mybir.AxisListType.X)
                vector.drain()
                vector.reciprocal(inv_s[:, :], s4[:, :])
                vector.drain().then_inc(esem, 1)  # ->1
                vector.wait_ge(esem, e0 + 2)
                vector.tensor_tensor(p_k[:, :], ek[:, :], inv_s[:, :], OP.mult)
                vector.tensor_tensor(p_j[:, :], r4[:, :], inv_s[:, :], OP.mult)
                vector.drain()
                vector.tensor_tensor(ptn[:, :], p_j[:, :], p_k[:, :], OP.subtract)
                vector.drain()
                vector.tensor_scalar(q_t[:, :], ptn[:, :], 1.0, None, OP.add)
                vector.tensor_scalar(pt[:, :], ptn[:, :], -1.0, None, OP.mult)
                # gamma = g0 + sum_b dg_b * [pt >= ub_b]
                vector.tensor_scalar(gam[:, :], pt[:, :], 0.0, g0[:, 0:1],
                                     OP.mult, OP.add)
                vector.drain()
                for b in range(NBINS - 1):
                    vector.tensor_scalar(tmp[:, :], pt[:, :], ub[:, b:b + 1],
                                         dg[:, b:b + 1], OP.is_ge, OP.mult)
                    vector.drain()
                    vector.tensor_tensor(gam[:, :], gam[:, :], tmp[:, :], OP.add)
                    vector.drain()
                vector.drain().then_inc(esem, 1)  # ->3
                vector.wait_ge(esem, e0 + 4)
                vector.tensor_tensor(gl[:, :], gam[:, :], lq[:, :], OP.mult)
                vector.drain().then_inc(esem, 1)  # ->5
                vector.wait_ge(esem, e0 + 6)
                vector.tensor_tensor(nlp[:, :], ls[:, :], xk[:, :], OP.subtract)
                vector.drain()
                vector.tensor_tensor(out_t[:, 0:NT], pw[:, :], nlp[:, :], OP.mult)
                vector.tensor_copy(out_t[:, NT:2 * NT], s4[:, :])
                vector.tensor_copy(out_t[:, 2 * NT:3 * NT], r4[:, :])
                if debug:
                    for j, t in enumerate([p_k, p_j, q_t, pt, gam, lq, pw, ls]):
                        vector.tensor_copy(out_t[:, (3 + j) * NT:(4 + j) * NT], t[:, :])
                vector.drain().then_inc(esem, 1)  # ->7

    return nc


def _prepare(input, target, bin_uppers, bin_gammas):
    input = np.asarray(input, dtype=np.float32)
    target = np.asarray(target, dtype=np.int32)
    bu = np.asarray(bin_uppers, dtype=np.float32)
    bg = np.asarray(bin_gammas, dtype=np.float32)

    xk_full = np.take_along_axis(input, target[:, None].astype(np.int64), axis=1)[:, 0]
    ub_b = np.ascontiguousarray(np.broadcast_to(bu[:NBINS - 1], (P, NBINS - 1)))
    g0_b = np.full((P, 1), bg[0], dtype=np.float32)
    dg_b = np.ascontiguousarray(
        np.broadcast_to(bg[1:] - bg[:-1], (P, NBINS - 1))).astype(np.float32)

    in_maps = []
    for i in range(NCORES):
        shard = np.ascontiguousarray(input[i * RPC:(i + 1) * RPC])
        xk_i = np.ascontiguousarray(
            xk_full[i * RPC:(i + 1) * RPC].reshape(NT, P).T).astype(np.float32)
        in_maps.append({"input": shard, "xk": xk_i, "ub": ub_b,
                        "g0": g0_b, "dg": dg_b})
    return in_maps


def kernel(input, target, bin_uppers, bin_gammas):
    global LAST_EXEC_NS
    if "nc" not in _CACHE:
        _CACHE["nc"] = build()
    nc = _CACHE["nc"]
    in_maps = _prepare(input, target, bin_uppers, bin_gammas)
    trace = bool(int(os.environ.get("ADK_TRACE", "0")))
    res = run_bass_kernel_spmd(nc, in_maps, core_ids=list(range(NCORES)),
                               trace=trace)
    LAST_EXEC_NS = res.exec_time_ns
    tot = 0.0
    for i in range(NCORES):
        tot += float(res.results[i]["out"][:, 0:NT].sum(dtype=np.float64))
    return np.float32(tot)
